# revision 25
# baseline (speedup 1.0000x reference)
"""Trainium2 Bass kernel for nn_MeshDeformationBlock (GNN message passing).

Data-parallel over batch: 2 batches per core, 8 cores.  Math rewrite:
  out = g@P0 + (A g)@P1 + (A^2 g)@P2 + (A^3 g)@P3      (biases are zero)
with g = bilinear(img, pos) + vertex_padded, A the symmetric edge operator,
P0..P3 host-precomputed 128x128 weight products.

Layout: vertices sorted by (low-nbr-count, high-nbr-count) into uniform
256-slot chunks; gathers batched into ~4096-token granules round-robined
over 4 SWDGE queues (one queue per in-flight granule — concurrent
transpose-gathers race on HW, so none are used).  Bilinear uses a
host-built 4-pixel table (one 1KB token per vertex) with compact
per-vertex weights broadcast on-chip via stride-0 APs.  Every state
table is mirrored channel-major ([2,128,Vp]) at production time via DVE
32x32 stream-transposes + block-permuting stores on the idle
Scalar/Sync DMA queues, so the final combine is plain contiguous loads
feeding PE matmuls with fp32 PSUM accumulation.
"""

import sys
import numpy as np
import ml_dtypes

sys.path.insert(0, "/opt/trn_rl_repo")

bf16 = ml_dtypes.bfloat16

B, V, C, H, W = 16, 40000, 128, 56, 56
NCORES = 8
NB = 2
CS = 256          # chunk slots
NVB = CS // 128
GR_CAP = 2048     # max tokens per gather granule
CVB = 2048        # bilinear block rows
NPIX = H * W


# ---------------------------------------------------------------- host plan

def _build_graph_plan(edges):
    e = np.asarray(edges).astype(np.int64)
    src = np.concatenate([e[:, 1], e[:, 0]])
    dst = np.concatenate([e[:, 0], e[:, 1]])
    deg = np.bincount(dst, minlength=V).astype(np.int64)

    order = np.argsort(dst, kind="stable")
    nbr_flat = src[order]
    rowptr = np.zeros(V + 1, np.int64)
    rowptr[1:] = np.cumsum(deg)

    counts_by_d = np.bincount(deg)
    cum = np.cumsum(counts_by_d)
    dstar = int(np.searchsorted(cum, 18000))
    halfbit = deg <= dstar

    a_of = np.zeros(V, np.int64)
    np.add.at(a_of, dst[order], halfbit[nbr_flat].astype(np.int64))
    b_of = deg - a_of

    chunks = []       # (base, A, B, n_real)
    rowpos = np.full(V, -1, np.int64)
    chunk_slot_vs = []
    pos = 0
    half = None
    for side in (0, 1):
        # leading all-zero chunk per half: dummy/padding tokens point at its
        # first row, so their contributions vanish.
        chunks.append((pos, 0, 0, 0))
        chunk_slot_vs.append(np.zeros(0, np.int64))
        pos += CS
        vs = np.nonzero(halfbit if side == 0 else ~halfbit)[0]
        o = np.lexsort((b_of[vs], a_of[vs]))
        vs = vs[o]
        n = len(vs)
        for i in range(0, n, CS):
            cvs = vs[i:i + CS]
            q = np.arange(len(cvs))
            rowpos[cvs] = pos + (q % 128) * NVB + q // 128
            chunks.append((pos, int(a_of[cvs].max()), int(b_of[cvs].max()),
                           len(cvs)))
            chunk_slot_vs.append(cvs)
            pos += CS
        if side == 0:
            half = pos
    assert half is not None and half < 32768 and (pos - half) < 32768
    Vp = -(-pos // 512) * 512
    if Vp > pos:
        chunks.append((pos, 0, 0, 0))
        chunk_slot_vs.append(np.zeros(0, np.int64))
        pos = Vp

    low_nbrs, high_nbrs = {}, {}
    for v in range(V):
        ns = nbr_flat[rowptr[v]:rowptr[v + 1]]
        lb = halfbit[ns]
        low_nbrs[v] = rowpos[ns[lb]]
        high_nbrs[v] = rowpos[ns[~lb]] - half

    # token streams + per-stream granule packing.  A granule is one gather
    # call (<= GR_CAP tokens); chunk blocks never straddle granules.
    granules = []              # (aidx_off, ntok)
    tok_parts = []
    gmap = {}                  # (stream, ci) -> (granule_id, off_in_granule)
    cur_items = {0: [], 1: []}
    cur_toks = {0: [], 1: []}
    cur_sz = {0: 0, 1: 0}

    def close(stream):
        if not cur_sz[stream]:
            return
        off = sum(len(t) for t in tok_parts)
        gi = len(granules)
        granules.append((off, cur_sz[stream]))
        tok_parts.extend(cur_toks[stream])
        for ci, off_in in cur_items[stream]:
            gmap[(stream, ci)] = (gi, off_in)
        cur_items[stream], cur_toks[stream] = [], []
        cur_sz[stream] = 0

    for ci, (base, A, Bn, nreal) in enumerate(chunks):
        cvs = chunk_slot_vs[ci]
        for stream, D, nbrs in ((0, A, low_nbrs), (1, Bn, high_nbrs)):
            if D == 0:
                continue
            blk = np.zeros((D, CS), np.int64)
            for q, v in enumerate(cvs):
                r = nbrs[v]
                blk[: len(r), q] = r
            if cur_sz[stream] and cur_sz[stream] + D * CS > GR_CAP:
                close(stream)
            cur_items[stream].append((ci, cur_sz[stream]))
            cur_toks[stream].append(blk.reshape(-1))
            cur_sz[stream] += D * CS
            if cur_sz[stream] >= GR_CAP:
                close(stream)
    close(0)
    close(1)

    tok = (np.concatenate(tok_parts) if tok_parts else np.zeros(0, np.int64))
    assert len(tok) % 128 == 0
    tok = tok.astype(np.int16)

    chunk_meta = []
    for ci, (base, A, Bn, nreal) in enumerate(chunks):
        lo = gmap.get((0, ci))
        hi = gmap.get((1, ci))
        chunk_meta.append((base, A, Bn,
                           lo[0] if lo else -1, lo[1] if lo else 0,
                           hi[0] if hi else -1, hi[1] if hi else 0))

    return dict(rowpos=rowpos, Vp=Vp, half=half, chunks=chunk_meta,
                granules=granules, tok=tok)


def _wrap16(stream):
    n = len(stream)
    assert n % 16 == 0
    w = stream.reshape(n // 16, 16).T
    return np.ascontiguousarray(np.tile(w, (8, 1))).astype(np.int16)


def _bilinear_host(plan, pos_b):
    """Per-batch: pixel-table token stream (block-colmajor order) and compact
    4-tap weights [Vp, 4] in storage-row order."""
    Vp = plan["Vp"]
    rowpos = plan["rowpos"]
    x = (pos_b[:, 0] + 1.0) * 0.5 * (W - 1)
    y = (pos_b[:, 1] + 1.0) * 0.5 * (H - 1)
    x0 = np.floor(x)
    y0 = np.floor(y)
    wx1 = (x - x0).astype(np.float32)
    wx0 = 1.0 - wx1
    wy1 = (y - y0).astype(np.float32)
    wy0 = 1.0 - wy1
    x0 = np.clip(x0.astype(np.int64), 0, W - 1)
    y0 = np.clip(y0.astype(np.int64), 0, H - 1)

    pixidx = np.zeros(Vp, np.int64)
    w4 = np.zeros((Vp, 4), np.float32)
    pixidx[rowpos] = y0 * W + x0
    w4[rowpos, 0] = wx0 * wy0
    w4[rowpos, 1] = wx1 * wy0
    w4[rowpos, 2] = wx0 * wy1
    w4[rowpos, 3] = wx1 * wy1

    stream = []
    for r0 in range(0, Vp, CVB):
        cv = min(CVB, Vp - r0)
        nv = cv // 128
        t = np.arange(cv)
        rows = r0 + (t % 128) * nv + t // 128
        stream.append(pixidx[rows])
    stream = np.concatenate(stream).astype(np.int16)
    return _wrap16(stream), w4.astype(bf16)


# ---------------------------------------------------------------- device

def _build_kernel(plan):
    import concourse.bacc as bacc
    import concourse.mybir as mybir
    from concourse.tile import TileContext

    Vp, half = plan["Vp"], plan["half"]
    chunks = plan["chunks"]
    granules = plan["granules"]
    TOK = len(plan["tok"])

    nc = bacc.Bacc("TRN2", target_bir_lowering=False, debug=False,
                   num_swdge_queues=4)
    dt = mybir.dt

    ident = nc.dram_tensor("ident", [128, 128], dt.bfloat16,
                           kind="ExternalInput")
    img4 = nc.dram_tensor("img4", [NB, NPIX, 512], dt.bfloat16,
                          kind="ExternalInput")
    vpadp = nc.dram_tensor("vpadp", [Vp, 256], dt.bfloat16,
                           kind="ExternalInput")
    bidx = nc.dram_tensor("bidx", [NB, 128, Vp // 16], dt.int16,
                          kind="ExternalInput")
    w4t = nc.dram_tensor("w4t", [NB, Vp, 4], dt.bfloat16,
                         kind="ExternalInput")
    aidx = nc.dram_tensor("aidx", [128, TOK // 16], dt.int16,
                          kind="ExternalInput")
    Pmat = nc.dram_tensor("Pmat", [4, 128, 128], dt.bfloat16,
                          kind="ExternalInput")
    outcm = nc.dram_tensor("outcm", [NB, 128, Vp], dt.float32,
                           kind="ExternalOutput")

    g_t = nc.dram_tensor("g_t", [Vp, 256], dt.bfloat16)
    a_t = [nc.dram_tensor(f"a{r}_t", [Vp, 256], dt.bfloat16)
           for r in range(3)]
    # channel-major mirrors: xc[k][x, c, row] = table_k[row, x*128+c]
    xc = [nc.dram_tensor(f"xc{k}", [2, 128, Vp], dt.bfloat16)
          for k in range(4)]

    def cm(dram_rows):
        return dram_rows.rearrange("(p u) e -> p u e", p=128)

    qn = [0]
    with TileContext(nc) as tc:
        with tc.tile_pool(name="res", bufs=1) as res:
            aidx_sb = res.tile([128, TOK // 16], dt.int16)
            nc.sync.dma_start(out=aidx_sb[:], in_=aidx[:, :])
            P_sb = res.tile([128, 4, 128], dt.bfloat16)
            nc.sync.dma_start(out=P_sb[:],
                              in_=Pmat[:, :, :].rearrange("k p m -> p k m"))
            id_sb = res.tile([128, 128], dt.bfloat16)
            nc.sync.dma_start(out=id_sb[:], in_=ident[:, :])
            zt = res.tile([128, NVB, 256], dt.bfloat16)
            nc.vector.memset(zt[:], 0.0)

            # ------------- phase B: g = bilinear + vpad -------------
            with (tc.tile_pool(name="bil", bufs=2) as bilp,
                  tc.tile_pool(name="bidxp", bufs=1) as bidxp):
                bidx_sb = []
                for b in range(NB):
                    t = bidxp.tile([128, Vp // 16], dt.int16, tag=f"bi{b}")
                    nc.sync.dma_start(out=t[:], in_=bidx[b, :, :])
                    bidx_sb.append(t)
                for r0 in range(0, Vp, CVB):
                    cv = min(CVB, Vp - r0)
                    nv = cv // 128
                    gst = bilp.tile([128, nv, 2, 128], dt.bfloat16, tag="gst")
                    vp = bilp.tile([128, nv, 2, 128], dt.bfloat16, tag="vp")
                    nc.sync.dma_start(
                        out=vp[:],
                        in_=cm(vpadp[r0:r0 + cv, :])
                        .rearrange("p u (x c) -> p u x c", x=2))
                    for b in range(NB):
                        taps = bilp.tile([128, nv, 4, 128], dt.bfloat16,
                                         tag=f"taps{b}")
                        nc.gpsimd.dma_gather(
                            taps[:].rearrange("p u x c -> p u (x c)"),
                            img4[b, :, :],
                            bidx_sb[b][:, r0 // 16:(r0 + cv) // 16],
                            cv, cv, 512, single_packet=False,
                            queue_num=qn[0] % 4)
                        qn[0] += 1
                        w4sb = bilp.tile([128, nv, 4], dt.bfloat16,
                                         tag=f"w4{b}")
                        nc.sync.dma_start(out=w4sb[:],
                                          in_=cm(w4t[b, r0:r0 + cv, :]))
                        w4b = (w4sb[:].rearrange("p u x -> p (u x)")
                               .unsqueeze(2).broadcast_to((128, nv * 4, 128)))
                        t3 = taps[:].rearrange("p u x c -> p (u x) c")
                        nc.vector.tensor_mul(out=t3, in0=t3, in1=w4b)
                        tf = taps[:].rearrange("p u x c -> p u (x c)")
                        nc.vector.tensor_add(out=tf[:, :, 0:256],
                                             in0=tf[:, :, 0:256],
                                             in1=tf[:, :, 256:512])
                        nc.vector.tensor_add(out=gst[:, :, b, :],
                                             in0=tf[:, :, 0:128],
                                             in1=tf[:, :, 128:256])
                        nc.vector.tensor_add(out=gst[:, :, b, :],
                                             in0=gst[:, :, b, :],
                                             in1=vp[:, :, b, :])
                    nc.sync.dma_start(
                        out=cm(g_t.ap()[r0:r0 + cv, :]),
                        in_=gst[:].rearrange("p u x c -> p u (x c)"))

            # ------------- phase C: a_{r+1} = A a_r -----------------
            # Rounds only gather + DVE-accumulate + store vertex-major, so
            # the SWDGE queues free-run.  The channel-major mirror of table
            # k (ready before round k starts) is built by an independent
            # "transpose pass" whose blocks are interleaved through the
            # round: 2048-row load -> 32 PE transposes -> 32 scalar copies
            # into a staging tile -> one big store.  a2's pass is fused
            # into phase D.
            GRB = GR_CAP // 128
            GRBL = max(gn for _, gn in granules) // 128
            PB = 2048
            PNV = PB // 128

            def pass_block(pool, tpp, src_vm, dcm, g0):
                # u-major load (row = g0 + u*128 + p): transposed pane (u,x)
                # covers columns [128u, 128u+128) -> contiguous copies.
                pl = pool.tile([128, PNV, 2, 128], dt.bfloat16, tag="pl")
                nc.sync.dma_start(
                    out=pl[:],
                    in_=src_vm.ap()[g0:g0 + PB, :]
                    .rearrange("(u p) (x c) -> p u x c", p=128, x=2))
                pst = pool.tile([128, 2, PB], dt.bfloat16, tag="pst")
                for u in range(PNV):
                    for x in range(2):
                        tp = tpp.tile([128, 128], dt.bfloat16, tag="tp")
                        nc.tensor.transpose(tp[:], pl[:, u, x, :], id_sb[:])
                        nc.scalar.activation(
                            out=pst[:, x, 128 * u:128 * (u + 1)], in_=tp[:],
                            func=mybir.ActivationFunctionType.Copy)
                nc.sync.dma_start(
                    out=dcm.ap()[:, :, g0:g0 + PB]
                    .rearrange("x c n -> c x n"),
                    in_=pst[:])

            with (tc.tile_pool(name="gb", bufs=10) as gbp,
                  tc.tile_pool(name="tp", bufs=8, space="PSUM") as tpp,
                  tc.tile_pool(name="pp", bufs=2) as passp,
                  tc.tile_pool(name="ac", bufs=8) as acp):
                for r in range(3):
                    src = g_t if r == 0 else a_t[r - 1]
                    dst = a_t[r]
                    issued = {}
                    ci = 0
                    pass_g0 = 0
                    for (base, A, Bn, lo_g, lo_off, hi_g, hi_off) in chunks:
                        ci += 1
                        if ci % 8 == 4 and pass_g0 < Vp:
                            pass_block(passp, tpp, src, xc[r], pass_g0)
                            pass_g0 += PB
                        for gidx, is_hi in ((lo_g, 0), (hi_g, 1)):
                            if gidx < 0 or gidx in issued:
                                continue
                            goff, gn = granules[gidx]
                            big = gn > GR_CAP
                            buf = gbp.tile(
                                [128, (GRBL if big else GRB), 256],
                                dt.bfloat16, tag=("gbL" if big else "gb"),
                                bufs=(2 if big else 10))
                            src_ap = (src.ap()[half:, :] if is_hi
                                      else src.ap()[:, :])
                            nc.gpsimd.dma_gather(
                                buf[:, :gn // 128, :], src_ap,
                                aidx_sb[:, goff // 16:(goff + gn) // 16],
                                gn, gn, 256, single_packet=False,
                                queue_num=qn[0] % 4)
                            qn[0] += 1
                            issued[gidx] = buf
                        D = A + Bn
                        if D == 0:
                            nc.sync.dma_start(
                                out=cm(dst.ap()[base:base + CS, :]),
                                in_=zt[:])
                            continue
                        slices = []
                        if A:
                            bl = issued[lo_g]
                            o = lo_off // 128
                            slices += [bl[:, o + k * NVB:o + (k + 1) * NVB, :]
                                       for k in range(A)]
                        if Bn:
                            bh = issued[hi_g]
                            o = hi_off // 128
                            slices += [bh[:, o + k * NVB:o + (k + 1) * NVB, :]
                                       for k in range(Bn)]
                        if D == 1:
                            nc.sync.dma_start(
                                out=cm(dst.ap()[base:base + CS, :]),
                                in_=slices[0])
                            continue
                        acct = acp.tile([128, NVB, 256], dt.bfloat16,
                                        tag="acc")
                        nc.vector.tensor_add(out=acct[:], in0=slices[0],
                                             in1=slices[1])
                        for s in slices[2:]:
                            nc.vector.tensor_add(out=acct[:], in0=acct[:],
                                                 in1=s)
                        nc.sync.dma_start(
                            out=cm(dst.ap()[base:base + CS, :]), in_=acct[:])
                    while pass_g0 < Vp:
                        pass_block(passp, tpp, src, xc[r], pass_g0)
                        pass_g0 += PB

            # ------------- phase D: combine (fused with a2 pass) ----
            with (tc.tile_pool(name="dp", bufs=3) as dp,
                  tc.tile_pool(name="tp2", bufs=4, space="PSUM") as tpp2,
                  tc.tile_pool(name="ps", bufs=4, space="PSUM") as psp):
                for g0 in range(0, Vp, PB):
                    al = dp.tile([128, PNV, 2, 128], dt.bfloat16, tag="al")
                    nc.sync.dma_start(
                        out=al[:],
                        in_=a_t[2].ap()[g0:g0 + PB, :]
                        .rearrange("(u p) (x c) -> p u x c", p=128, x=2))
                    dstage = dp.tile([128, 2, PB], dt.bfloat16, tag="dst")
                    for u in range(PNV):
                        for x in range(2):
                            tp = tpp2.tile([128, 128], dt.bfloat16, tag="tp2")
                            nc.tensor.transpose(tp[:], al[:, u, x, :],
                                                id_sb[:])
                            ocols = dstage[:, x, 128 * u:128 * (u + 1)]
                            if (u + x) % 2:
                                nc.vector.tensor_copy(out=ocols, in_=tp[:])
                            else:
                                nc.scalar.activation(
                                    out=ocols, in_=tp[:],
                                    func=mybir.ActivationFunctionType.Copy)
                    for sub in range(PB // 512):
                        blk = g0 + 512 * sub
                        for b in range(NB):
                            ps = psp.tile([128, 512], dt.float32)
                            for k in range(3):
                                xb = dp.tile([128, 512], dt.bfloat16,
                                             tag=f"x{k}b{b}")
                                eng = nc.sync if k < 2 else nc.scalar
                                eng.dma_start(
                                    out=xb[:], in_=xc[k][b, :, blk:blk + 512])
                                nc.tensor.matmul(out=ps[:],
                                                 lhsT=P_sb[:, k, :],
                                                 rhs=xb[:],
                                                 start=(k == 0), stop=False)
                            nc.tensor.matmul(
                                out=ps[:], lhsT=P_sb[:, 3, :],
                                rhs=dstage[:, b, 512 * sub:512 * (sub + 1)],
                                start=False, stop=True)
                            ot = dp.tile([128, 512], dt.float32, tag=f"ot{b}")
                            nc.scalar.activation(
                                out=ot[:], in_=ps[:],
                                func=mybir.ActivationFunctionType.Copy)
                            nc.sync.dma_start(out=outcm[b, :, blk:blk + 512],
                                              in_=ot[:])

    nc.compile()
    return nc


# ---------------------------------------------------------------- entry

def _make_in_maps(plan, inputs):
    Vp = plan["Vp"]
    rowpos = plan["rowpos"]
    M = [np.asarray(inputs[f"w0_{i}"], np.float64) for i in (1, 2, 3)]
    N = [np.asarray(inputs[f"w1_{i}"], np.float64) for i in (1, 2, 3)]
    P0 = M[0] + M[0] @ M[1] @ M[2]
    P1 = N[0] + N[0] @ M[1] @ M[2] + M[0] @ N[1] @ M[2] + M[0] @ M[1] @ N[2]
    P2 = N[0] @ N[1] @ M[2] + N[0] @ M[1] @ N[2] + M[0] @ N[1] @ N[2]
    P3 = N[0] @ N[1] @ N[2]
    Pm = np.ascontiguousarray(np.stack([P0, P1, P2, P3]).astype(bf16))

    img = np.asarray(inputs["img_features"], np.float32)
    pos = np.asarray(inputs["vertex_position"], np.float32)
    vpad = np.asarray(inputs["vertex_padded"], np.float32)

    # 4-pixel table per batch: row(y*W+x) = [f(y,x), f(y,x+1), f(y+1,x),
    # f(y+1,x+1)] with clamped borders (their taps always carry weight 0).
    F = img.transpose(0, 2, 3, 1)  # [B, H, W, C]
    ys, xs = np.mgrid[0:H, 0:W]
    yp = np.minimum(ys + 1, H - 1)
    xp = np.minimum(xs + 1, W - 1)
    img4_all = np.concatenate(
        [F[:, ys, xs], F[:, ys, xp], F[:, yp, xs], F[:, yp, xp]],
        axis=-1).reshape(B, NPIX, 512).astype(bf16)

    aidx_w = _wrap16(plan["tok"])

    in_maps = []
    for core in range(NCORES):
        bs = [NB * core + i for i in range(NB)]
        bidx_l, w4_l = [], []
        for b in bs:
            bi, w4 = _bilinear_host(plan, pos[b])
            bidx_l.append(bi)
            w4_l.append(w4)
        vpadp = np.zeros((Vp, 2, 128), bf16)
        for i, b in enumerate(bs):
            vpadp[rowpos, i, :] = vpad[b].astype(bf16)
        in_maps.append({
            "ident": np.eye(128, dtype=np.float32).astype(bf16),
            "img4": np.ascontiguousarray(img4_all[bs]),
            "vpadp": np.ascontiguousarray(vpadp.reshape(Vp, 256)),
            "bidx": np.ascontiguousarray(np.stack(bidx_l)),
            "w4t": np.ascontiguousarray(np.stack(w4_l)),
            "aidx": aidx_w,
            "Pmat": Pm,
        })
    return in_maps


_CACHE = {}


def kernel(**inputs):
    from concourse import bass_utils

    plan = _build_graph_plan(inputs["edges"])
    in_maps = _make_in_maps(plan, inputs)
    key = "nc"
    if key not in _CACHE:
        _CACHE[key] = _build_kernel(plan)
    nc = _CACHE[key]
    res = bass_utils.run_bass_kernel_spmd(nc, in_maps,
                                          core_ids=list(range(NCORES)))

    out = np.zeros((B, V, C), np.float32)
    for core in range(NCORES):
        oc = res.results[core]["outcm"]
        for i in range(NB):
            out[NB * core + i] = oc[i][:, plan["rowpos"]].T
    return out


# revision 26
# speedup vs baseline: 1.2753x; 1.2753x over previous
"""Trainium2 Bass kernel for nn_MeshDeformationBlock (GNN message passing).

Data-parallel over batch: 2 batches per core, 8 cores.  Math rewrite:
  out = g@P0 + (A g)@P1 + (A^2 g)@P2 + (A^3 g)@P3      (biases are zero)
with g = bilinear(img, pos) + vertex_padded, A the symmetric edge operator,
P0..P3 host-precomputed 128x128 weight products.

Layout: vertices sorted by (low-nbr-count, high-nbr-count) into uniform
256-slot chunks; gathers batched into ~4096-token granules round-robined
over 4 SWDGE queues (one queue per in-flight granule — concurrent
transpose-gathers race on HW, so none are used).  Bilinear uses a
host-built 4-pixel table (one 1KB token per vertex) with compact
per-vertex weights broadcast on-chip via stride-0 APs.  Every state
table is mirrored channel-major ([2,128,Vp]) at production time via DVE
32x32 stream-transposes + block-permuting stores on the idle
Scalar/Sync DMA queues, so the final combine is plain contiguous loads
feeding PE matmuls with fp32 PSUM accumulation.
"""

import sys
import numpy as np
import ml_dtypes

sys.path.insert(0, "/opt/trn_rl_repo")

bf16 = ml_dtypes.bfloat16

B, V, C, H, W = 16, 40000, 128, 56, 56
NCORES = 8
NB = 2
CS = 256          # chunk slots
NVB = CS // 128
GR_CAP = 2048     # max tokens per gather granule
CVB = 2048        # bilinear block rows
NPIX = H * W


# ---------------------------------------------------------------- host plan

def _build_graph_plan(edges):
    e = np.asarray(edges).astype(np.int64)
    src = np.concatenate([e[:, 1], e[:, 0]])
    dst = np.concatenate([e[:, 0], e[:, 1]])
    deg = np.bincount(dst, minlength=V).astype(np.int64)

    order = np.argsort(dst, kind="stable")
    nbr_flat = src[order]
    rowptr = np.zeros(V + 1, np.int64)
    rowptr[1:] = np.cumsum(deg)

    counts_by_d = np.bincount(deg)
    cum = np.cumsum(counts_by_d)
    dstar = int(np.searchsorted(cum, 18000))
    halfbit = deg <= dstar

    a_of = np.zeros(V, np.int64)
    np.add.at(a_of, dst[order], halfbit[nbr_flat].astype(np.int64))
    b_of = deg - a_of

    chunks = []       # (base, A, B, n_real)
    rowpos = np.full(V, -1, np.int64)
    chunk_slot_vs = []
    pos = 0
    half = None
    for side in (0, 1):
        # leading all-zero chunk per half: dummy/padding tokens point at its
        # first row, so their contributions vanish.
        chunks.append((pos, 0, 0, 0))
        chunk_slot_vs.append(np.zeros(0, np.int64))
        pos += CS
        vs = np.nonzero(halfbit if side == 0 else ~halfbit)[0]
        o = np.lexsort((b_of[vs], a_of[vs]))
        vs = vs[o]
        n = len(vs)
        for i in range(0, n, CS):
            cvs = vs[i:i + CS]
            q = np.arange(len(cvs))
            rowpos[cvs] = pos + (q % 128) * NVB + q // 128
            chunks.append((pos, int(a_of[cvs].max()), int(b_of[cvs].max()),
                           len(cvs)))
            chunk_slot_vs.append(cvs)
            pos += CS
        if side == 0:
            half = pos
    assert half is not None and half < 32768 and (pos - half) < 32768
    Vp = -(-pos // 512) * 512
    if Vp > pos:
        chunks.append((pos, 0, 0, 0))
        chunk_slot_vs.append(np.zeros(0, np.int64))
        pos = Vp

    low_nbrs, high_nbrs = {}, {}
    for v in range(V):
        ns = nbr_flat[rowptr[v]:rowptr[v + 1]]
        lb = halfbit[ns]
        low_nbrs[v] = rowpos[ns[lb]]
        high_nbrs[v] = rowpos[ns[~lb]] - half

    # token streams + per-stream granule packing.  A granule is one gather
    # call (<= GR_CAP tokens); chunk blocks never straddle granules.
    granules = []              # (aidx_off, ntok)
    tok_parts = []
    gmap = {}                  # (stream, ci) -> (granule_id, off_in_granule)
    cur_items = {0: [], 1: []}
    cur_toks = {0: [], 1: []}
    cur_sz = {0: 0, 1: 0}

    def close(stream):
        if not cur_sz[stream]:
            return
        off = sum(len(t) for t in tok_parts)
        gi = len(granules)
        granules.append((off, cur_sz[stream]))
        tok_parts.extend(cur_toks[stream])
        for ci, off_in in cur_items[stream]:
            gmap[(stream, ci)] = (gi, off_in)
        cur_items[stream], cur_toks[stream] = [], []
        cur_sz[stream] = 0

    for ci, (base, A, Bn, nreal) in enumerate(chunks):
        cvs = chunk_slot_vs[ci]
        for stream, D, nbrs in ((0, A, low_nbrs), (1, Bn, high_nbrs)):
            if D == 0:
                continue
            blk = np.zeros((D, CS), np.int64)
            for q, v in enumerate(cvs):
                r = nbrs[v]
                blk[: len(r), q] = r
            if cur_sz[stream] and cur_sz[stream] + D * CS > GR_CAP:
                close(stream)
            cur_items[stream].append((ci, cur_sz[stream]))
            cur_toks[stream].append(blk.reshape(-1))
            cur_sz[stream] += D * CS
            if cur_sz[stream] >= GR_CAP:
                close(stream)
    close(0)
    close(1)

    tok = (np.concatenate(tok_parts) if tok_parts else np.zeros(0, np.int64))
    assert len(tok) % 128 == 0
    tok = tok.astype(np.int16)

    chunk_meta = []
    for ci, (base, A, Bn, nreal) in enumerate(chunks):
        lo = gmap.get((0, ci))
        hi = gmap.get((1, ci))
        chunk_meta.append((base, A, Bn,
                           lo[0] if lo else -1, lo[1] if lo else 0,
                           hi[0] if hi else -1, hi[1] if hi else 0))

    return dict(rowpos=rowpos, Vp=Vp, half=half, chunks=chunk_meta,
                granules=granules, tok=tok)


def _wrap16(stream):
    n = len(stream)
    assert n % 16 == 0
    w = stream.reshape(n // 16, 16).T
    return np.ascontiguousarray(np.tile(w, (8, 1))).astype(np.int16)


def _bilinear_host(plan, pos_b):
    """Per-batch: pixel-table token stream (block-colmajor order) and compact
    4-tap weights [Vp, 4] in storage-row order."""
    Vp = plan["Vp"]
    rowpos = plan["rowpos"]
    x = (pos_b[:, 0] + 1.0) * 0.5 * (W - 1)
    y = (pos_b[:, 1] + 1.0) * 0.5 * (H - 1)
    x0 = np.floor(x)
    y0 = np.floor(y)
    wx1 = (x - x0).astype(np.float32)
    wx0 = 1.0 - wx1
    wy1 = (y - y0).astype(np.float32)
    wy0 = 1.0 - wy1
    x0 = np.clip(x0.astype(np.int64), 0, W - 1)
    y0 = np.clip(y0.astype(np.int64), 0, H - 1)

    pixidx = np.zeros(Vp, np.int64)
    w4 = np.zeros((Vp, 4), np.float32)
    pixidx[rowpos] = y0 * W + x0
    w4[rowpos, 0] = wx0 * wy0
    w4[rowpos, 1] = wx1 * wy0
    w4[rowpos, 2] = wx0 * wy1
    w4[rowpos, 3] = wx1 * wy1

    stream = []
    for r0 in range(0, Vp, CVB):
        cv = min(CVB, Vp - r0)
        nv = cv // 128
        t = np.arange(cv)
        rows = r0 + (t % 128) * nv + t // 128
        stream.append(pixidx[rows])
    stream = np.concatenate(stream).astype(np.int16)
    return _wrap16(stream), w4.astype(bf16)


# ---------------------------------------------------------------- device

def _build_kernel(plan):
    import concourse.bacc as bacc
    import concourse.mybir as mybir
    from concourse.tile import TileContext

    Vp, half = plan["Vp"], plan["half"]
    chunks = plan["chunks"]
    granules = plan["granules"]
    TOK = len(plan["tok"])

    nc = bacc.Bacc("TRN2", target_bir_lowering=False, debug=False,
                   num_swdge_queues=4)
    dt = mybir.dt

    ident = nc.dram_tensor("ident", [128, 128], dt.bfloat16,
                           kind="ExternalInput")
    img4 = nc.dram_tensor("img4", [NB, NPIX, 512], dt.bfloat16,
                          kind="ExternalInput")
    vpadp = nc.dram_tensor("vpadp", [Vp, 256], dt.bfloat16,
                           kind="ExternalInput")
    bidx = nc.dram_tensor("bidx", [NB, 128, Vp // 16], dt.int16,
                          kind="ExternalInput")
    w4t = nc.dram_tensor("w4t", [NB, Vp, 4], dt.bfloat16,
                         kind="ExternalInput")
    aidx = nc.dram_tensor("aidx", [128, TOK // 16], dt.int16,
                          kind="ExternalInput")
    Pmat = nc.dram_tensor("Pmat", [4, 128, 128], dt.bfloat16,
                          kind="ExternalInput")
    outcm = nc.dram_tensor("outcm", [NB, 128, Vp], dt.float32,
                           kind="ExternalOutput")

    g_t = nc.dram_tensor("g_t", [Vp, 256], dt.bfloat16)
    a_t = [nc.dram_tensor(f"a{r}_t", [Vp, 256], dt.bfloat16)
           for r in range(3)]
    # channel-major mirrors: xc[k][x, c, row] = table_k[row, x*128+c]
    xc = [nc.dram_tensor(f"xc{k}", [2, 128, Vp], dt.bfloat16)
          for k in range(4)]

    def cm(dram_rows):
        return dram_rows.rearrange("(p u) e -> p u e", p=128)

    qn = [0]
    with TileContext(nc) as tc:
        with tc.tile_pool(name="res", bufs=1) as res:
            aidx_sb = res.tile([128, TOK // 16], dt.int16)
            nc.sync.dma_start(out=aidx_sb[:], in_=aidx[:, :])
            P_sb = res.tile([128, 4, 128], dt.bfloat16)
            nc.sync.dma_start(out=P_sb[:],
                              in_=Pmat[:, :, :].rearrange("k p m -> p k m"))
            id_sb = res.tile([128, 128], dt.bfloat16)
            nc.sync.dma_start(out=id_sb[:], in_=ident[:, :])
            zt = res.tile([128, NVB, 256], dt.bfloat16)
            nc.vector.memset(zt[:], 0.0)

            # ------------- phase B: g = bilinear + vpad -------------
            with (tc.tile_pool(name="bil", bufs=2) as bilp,
                  tc.tile_pool(name="bidxp", bufs=1) as bidxp):
                bidx_sb = []
                for b in range(NB):
                    t = bidxp.tile([128, Vp // 16], dt.int16, tag=f"bi{b}")
                    nc.sync.dma_start(out=t[:], in_=bidx[b, :, :])
                    bidx_sb.append(t)
                for r0 in range(0, Vp, CVB):
                    cv = min(CVB, Vp - r0)
                    nv = cv // 128
                    gst = bilp.tile([128, nv, 2, 128], dt.bfloat16, tag="gst")
                    vp = bilp.tile([128, nv, 2, 128], dt.bfloat16, tag="vp")
                    nc.sync.dma_start(
                        out=vp[:],
                        in_=cm(vpadp[r0:r0 + cv, :])
                        .rearrange("p u (x c) -> p u x c", x=2))
                    for b in range(NB):
                        taps = bilp.tile([128, nv, 4, 128], dt.bfloat16,
                                         tag=f"taps{b}")
                        nc.gpsimd.dma_gather(
                            taps[:].rearrange("p u x c -> p u (x c)"),
                            img4[b, :, :],
                            bidx_sb[b][:, r0 // 16:(r0 + cv) // 16],
                            cv, cv, 512, single_packet=False,
                            queue_num=qn[0] % 4)
                        qn[0] += 1
                        w4sb = bilp.tile([128, nv, 4], dt.bfloat16,
                                         tag=f"w4{b}")
                        nc.sync.dma_start(out=w4sb[:],
                                          in_=cm(w4t[b, r0:r0 + cv, :]))
                        w4b = (w4sb[:].rearrange("p u x -> p (u x)")
                               .unsqueeze(2).broadcast_to((128, nv * 4, 128)))
                        t3 = taps[:].rearrange("p u x c -> p (u x) c")
                        nc.vector.tensor_mul(out=t3, in0=t3, in1=w4b)
                        tf = taps[:].rearrange("p u x c -> p u (x c)")
                        nc.vector.tensor_add(out=tf[:, :, 0:256],
                                             in0=tf[:, :, 0:256],
                                             in1=tf[:, :, 256:512])
                        nc.vector.tensor_add(out=gst[:, :, b, :],
                                             in0=tf[:, :, 0:128],
                                             in1=tf[:, :, 128:256])
                        nc.vector.tensor_add(out=gst[:, :, b, :],
                                             in0=gst[:, :, b, :],
                                             in1=vp[:, :, b, :])
                    nc.sync.dma_start(
                        out=cm(g_t.ap()[r0:r0 + cv, :]),
                        in_=gst[:].rearrange("p u x c -> p u (x c)"))

            # ------------- phase C: a_{r+1} = A a_r -----------------
            # Rounds only gather + DVE-accumulate + store vertex-major, so
            # the SWDGE queues free-run.  The channel-major mirror of table
            # k (ready before round k starts) is built by an independent
            # "transpose pass" whose blocks are interleaved through the
            # round: 2048-row load -> 32 PE transposes -> 32 scalar copies
            # into a staging tile -> one big store.  a2's pass is fused
            # into phase D.
            GRB = GR_CAP // 128
            GRBL = max(gn for _, gn in granules) // 128
            PB = 2048
            SGN = 4
            PNV = PB // 128

            def pass_block(pool, tpp, src_vm, dcm, g0):
                # u-major load (row = g0 + u*128 + p): transposed pane (u,x)
                # covers columns [128u, 128u+128) -> contiguous copies.
                pl = pool.tile([128, PNV, 2, 128], dt.bfloat16, tag="pl")
                nc.sync.dma_start(
                    out=pl[:],
                    in_=src_vm.ap()[g0:g0 + PB, :]
                    .rearrange("(u p) (x c) -> p u x c", p=128, x=2))
                pst = pool.tile([128, 2, PB], dt.bfloat16, tag="pst")
                for u in range(PNV):
                    for x in range(2):
                        tp = tpp.tile([128, 128], dt.bfloat16, tag="tp")
                        nc.tensor.transpose(tp[:], pl[:, u, x, :], id_sb[:])
                        nc.scalar.activation(
                            out=pst[:, x, 128 * u:128 * (u + 1)], in_=tp[:],
                            func=mybir.ActivationFunctionType.Copy)
                nc.sync.dma_start(
                    out=dcm.ap()[:, :, g0:g0 + PB]
                    .rearrange("x c n -> c x n"),
                    in_=pst[:])

            with (tc.tile_pool(name="gb", bufs=10) as gbp,
                  tc.tile_pool(name="tp", bufs=8, space="PSUM") as tpp,
                  tc.tile_pool(name="pp", bufs=2) as passp,
                  tc.tile_pool(name="ac", bufs=8) as acp):
                for r in range(3):
                    src = g_t if r == 0 else a_t[r - 1]
                    dst = a_t[r]
                    issued = {}
                    ci = 0
                    pass_g0 = 0
                    for (base, A, Bn, lo_g, lo_off, hi_g, hi_off) in chunks:
                        ci += 1
                        if ci % 8 == 4 and pass_g0 < Vp:
                            pass_block(passp, tpp, src, xc[r], pass_g0)
                            pass_g0 += PB
                        for gidx, is_hi in ((lo_g, 0), (hi_g, 1)):
                            if gidx < 0 or gidx in issued:
                                continue
                            goff, gn = granules[gidx]
                            big = gn > GR_CAP
                            buf = gbp.tile(
                                [128, (GRBL if big else GRB), 256],
                                dt.bfloat16, tag=("gbL" if big else "gb"),
                                bufs=(2 if big else 10))
                            src_ap = (src.ap()[half:, :] if is_hi
                                      else src.ap()[:, :])
                            nc.gpsimd.dma_gather(
                                buf[:, :gn // 128, :], src_ap,
                                aidx_sb[:, goff // 16:(goff + gn) // 16],
                                gn, gn, 256, single_packet=False,
                                queue_num=qn[0] % 4)
                            qn[0] += 1
                            issued[gidx] = buf
                        # accumulate into a 4-chunk staging tile; one
                        # merged vertex-major store per group keeps the sync
                        # DMA queue short so granule buffers recycle fast.
                        if base % (SGN * CS) == 0:
                            vmst = acp.tile([128, SGN, NVB, 256],
                                            dt.bfloat16, tag="vst", bufs=4)
                        sj = (base % (SGN * CS)) // CS
                        dstv = vmst[:, sj, :, :]
                        D = A + Bn
                        slices = []
                        if A:
                            bl = issued[lo_g]
                            o = lo_off // 128
                            slices += [bl[:, o + k * NVB:o + (k + 1) * NVB, :]
                                       for k in range(A)]
                        if Bn:
                            bh = issued[hi_g]
                            o = hi_off // 128
                            slices += [bh[:, o + k * NVB:o + (k + 1) * NVB, :]
                                       for k in range(Bn)]
                        if D == 0:
                            nc.vector.tensor_copy(out=dstv, in_=zt[:])
                        elif D == 1:
                            nc.vector.tensor_copy(out=dstv, in_=slices[0])
                        else:
                            nc.vector.tensor_add(out=dstv, in0=slices[0],
                                                 in1=slices[1])
                            for s in slices[2:]:
                                nc.vector.tensor_add(out=dstv, in0=dstv,
                                                     in1=s)
                        if base % (SGN * CS) == (SGN - 1) * CS:
                            gbase = base + CS - SGN * CS
                            nc.sync.dma_start(
                                out=dst.ap()[gbase:gbase + SGN * CS, :]
                                .rearrange("(sj p j) e -> p sj (j e)",
                                           sj=SGN, p=128),
                                in_=vmst[:])
                    while pass_g0 < Vp:
                        pass_block(passp, tpp, src, xc[r], pass_g0)
                        pass_g0 += PB

            # ------------- phase D: combine (fused with a2 pass) ----
            with (tc.tile_pool(name="dp", bufs=3) as dp,
                  tc.tile_pool(name="tp2", bufs=4, space="PSUM") as tpp2,
                  tc.tile_pool(name="ps", bufs=4, space="PSUM") as psp):
                for g0 in range(0, Vp, PB):
                    al = dp.tile([128, PNV, 2, 128], dt.bfloat16, tag="al")
                    nc.sync.dma_start(
                        out=al[:],
                        in_=a_t[2].ap()[g0:g0 + PB, :]
                        .rearrange("(u p) (x c) -> p u x c", p=128, x=2))
                    dstage = dp.tile([128, 2, PB], dt.bfloat16, tag="dst")
                    for u in range(PNV):
                        for x in range(2):
                            tp = tpp2.tile([128, 128], dt.bfloat16, tag="tp2")
                            nc.tensor.transpose(tp[:], al[:, u, x, :],
                                                id_sb[:])
                            ocols = dstage[:, x, 128 * u:128 * (u + 1)]
                            if (u + x) % 2:
                                nc.vector.tensor_copy(out=ocols, in_=tp[:])
                            else:
                                nc.scalar.activation(
                                    out=ocols, in_=tp[:],
                                    func=mybir.ActivationFunctionType.Copy)
                    for sub in range(PB // 512):
                        blk = g0 + 512 * sub
                        for b in range(NB):
                            ps = psp.tile([128, 512], dt.float32)
                            for k in range(3):
                                xb = dp.tile([128, 512], dt.bfloat16,
                                             tag=f"x{k}b{b}")
                                eng = nc.sync if k < 2 else nc.scalar
                                eng.dma_start(
                                    out=xb[:], in_=xc[k][b, :, blk:blk + 512])
                                nc.tensor.matmul(out=ps[:],
                                                 lhsT=P_sb[:, k, :],
                                                 rhs=xb[:],
                                                 start=(k == 0), stop=False)
                            nc.tensor.matmul(
                                out=ps[:], lhsT=P_sb[:, 3, :],
                                rhs=dstage[:, b, 512 * sub:512 * (sub + 1)],
                                start=False, stop=True)
                            ot = dp.tile([128, 512], dt.float32, tag=f"ot{b}")
                            nc.scalar.activation(
                                out=ot[:], in_=ps[:],
                                func=mybir.ActivationFunctionType.Copy)
                            nc.sync.dma_start(out=outcm[b, :, blk:blk + 512],
                                              in_=ot[:])

    nc.compile()
    return nc


# ---------------------------------------------------------------- entry

def _make_in_maps(plan, inputs):
    Vp = plan["Vp"]
    rowpos = plan["rowpos"]
    M = [np.asarray(inputs[f"w0_{i}"], np.float64) for i in (1, 2, 3)]
    N = [np.asarray(inputs[f"w1_{i}"], np.float64) for i in (1, 2, 3)]
    P0 = M[0] + M[0] @ M[1] @ M[2]
    P1 = N[0] + N[0] @ M[1] @ M[2] + M[0] @ N[1] @ M[2] + M[0] @ M[1] @ N[2]
    P2 = N[0] @ N[1] @ M[2] + N[0] @ M[1] @ N[2] + M[0] @ N[1] @ N[2]
    P3 = N[0] @ N[1] @ N[2]
    Pm = np.ascontiguousarray(np.stack([P0, P1, P2, P3]).astype(bf16))

    img = np.asarray(inputs["img_features"], np.float32)
    pos = np.asarray(inputs["vertex_position"], np.float32)
    vpad = np.asarray(inputs["vertex_padded"], np.float32)

    # 4-pixel table per batch: row(y*W+x) = [f(y,x), f(y,x+1), f(y+1,x),
    # f(y+1,x+1)] with clamped borders (their taps always carry weight 0).
    F = img.transpose(0, 2, 3, 1)  # [B, H, W, C]
    ys, xs = np.mgrid[0:H, 0:W]
    yp = np.minimum(ys + 1, H - 1)
    xp = np.minimum(xs + 1, W - 1)
    img4_all = np.concatenate(
        [F[:, ys, xs], F[:, ys, xp], F[:, yp, xs], F[:, yp, xp]],
        axis=-1).reshape(B, NPIX, 512).astype(bf16)

    aidx_w = _wrap16(plan["tok"])

    in_maps = []
    for core in range(NCORES):
        bs = [NB * core + i for i in range(NB)]
        bidx_l, w4_l = [], []
        for b in bs:
            bi, w4 = _bilinear_host(plan, pos[b])
            bidx_l.append(bi)
            w4_l.append(w4)
        vpadp = np.zeros((Vp, 2, 128), bf16)
        for i, b in enumerate(bs):
            vpadp[rowpos, i, :] = vpad[b].astype(bf16)
        in_maps.append({
            "ident": np.eye(128, dtype=np.float32).astype(bf16),
            "img4": np.ascontiguousarray(img4_all[bs]),
            "vpadp": np.ascontiguousarray(vpadp.reshape(Vp, 256)),
            "bidx": np.ascontiguousarray(np.stack(bidx_l)),
            "w4t": np.ascontiguousarray(np.stack(w4_l)),
            "aidx": aidx_w,
            "Pmat": Pm,
        })
    return in_maps


_CACHE = {}


def kernel(**inputs):
    from concourse import bass_utils

    plan = _build_graph_plan(inputs["edges"])
    in_maps = _make_in_maps(plan, inputs)
    key = "nc"
    if key not in _CACHE:
        _CACHE[key] = _build_kernel(plan)
    nc = _CACHE[key]
    res = bass_utils.run_bass_kernel_spmd(nc, in_maps,
                                          core_ids=list(range(NCORES)))

    out = np.zeros((B, V, C), np.float32)
    for core in range(NCORES):
        oc = res.results[core]["outcm"]
        for i in range(NB):
            out[NB * core + i] = oc[i][:, plan["rowpos"]].T
    return out
